# revision 21
# baseline (speedup 1.0000x reference)
"""MoE FFN with auxiliary loss — Trainium2 Bass kernel.

Strategy (expert-parallel, host-routed):
  The reference computes every expert on every token, but the combine
  weights are nonzero only for each token's top-2 experts — so only
  top-2 expert outputs are needed.  Gating/top-k/aux-losses are tiny
  (0.03% of FLOPs) and run on host.  Each of the 8 NeuronCores owns one
  expert (E=8): the host gathers that expert's routed tokens, the core
  runs the SwiGLU FFN  (silu(x@W1^T) * (x@W2^T)) @ W3^T  on them, and
  the host scales by gate probs and scatters back.

  Device layout: everything transposed so the token axis is the matmul
  free (moving) axis — PSUM partition = output-feature tiles:
    phase 1: H1T[h,c] += W1T[d-tile,h-tile]^T @ XT[d-tile,c]   (lhsT=W1T)
    silu/mul fused on ACT+DVE into hidden slab [h, c] in SBUF
    phase 2: YT[d,c]  += W3T[h-tile,d-tile]^T @ hidden[h-tile,c]
  float32r matmuls run at full PE rate (1 cycle/row) for free dim >=256.
"""

import os
import sys

sys.path.insert(0, "/opt/trn_rl_repo")

last_results = None  # BassKernelResults of the most recent run (for profiling)
last_C = None  # capacity used by the most recent run

import numpy as np

import concourse.bass as bass
from concourse import bacc
import concourse.mybir as mybir
from concourse.bass_utils import run_bass_kernel_spmd
from concourse.tile import TileContext

TOP_K = 2
EPS = 1e-9
LB_COEF = 0.01
ENT_COEF = 0.0

B, S, D, E, H = 2, 2048, 1024, 8, 2048
N_CORES = 8
P = 128

F32R = mybir.dt.float32r
F32 = mybir.dt.float32


def _free_chunks(C):
    """Split C (>=512) into free-dim chunks, each in [256, 512] so float32r
    matmuls stay at the 1-cycle/row rate and PSUM free-dim fits."""
    chunks = []
    r = C
    while r > 512:
        take = 512 if r - 512 >= 256 else r - 256
        chunks.append(take)
        r -= take
    chunks.append(r)
    assert sum(chunks) == C and all(256 <= c <= 512 for c in chunks), (C, chunks)
    return chunks


def build_ffn_kernel(C, rounds=1):
    """One expert's SwiGLU FFN over C routed tokens (token axis = free).

    rounds>1 (benchmarking only) re-applies the FFN to its own output via
    ping-pong SBUF slabs, so per-round HW time can be isolated from the
    fixed dispatch overhead: (wall(R) - wall(1)) / (R - 1).
    """
    nc = bacc.Bacc()

    xt = nc.declare_dram_parameter("xt", [D, C], F32R, isOutput=False)
    w1t = nc.declare_dram_parameter("w1t", [D, H], F32R, isOutput=False)
    w2t = nc.declare_dram_parameter("w2t", [D, H], F32R, isOutput=False)
    w3t = nc.declare_dram_parameter("w3t", [H, D], F32R, isOutput=False)
    yt = nc.declare_dram_parameter("yt", [D, C], F32, isOutput=True)

    KD = D // P  # 8 k-tiles over D
    KH = H // P  # 16 tiles over H
    cks = _free_chunks(C)
    coff = np.cumsum([0] + cks)[:-1]

    with TileContext(nc) as tc:
        with (
            tc.tile_pool(name="resident", bufs=1) as resident,
            tc.tile_pool(name="wpool", bufs=3 if rounds == 1 else 2) as wpool,
            tc.tile_pool(name="act", bufs=3) as actp,
            tc.tile_pool(name="psum", bufs=2, space="PSUM") as psum,
        ):
            # resident slabs; rounds==1 writes back into xt_sb (phase 2 only
            # starts after phase 1 is done reading it), saving a third slab
            xt_sb = resident.tile([P, KD, C], F32R, tag="xt_sb")
            hid_sb = resident.tile([P, KH, C], F32R, tag="hid_sb")
            if rounds > 1:
                out_sb = resident.tile([P, KD, C], F32R, tag="out_sb")
            else:
                out_sb = xt_sb

            # load X^T: (k p) c -> p k c, two stages — chunk-0 columns for
            # all k first so the first matmuls start early, then the rest
            # (fully chunk-granular loads measured slower: DMA op overhead)
            xt_v = xt.rearrange("(k p) c -> p k c", p=P)
            c0w = cks[0]
            for k in range(KD):
                nc.sync.dma_start(out=xt_sb[:, k, :c0w], in_=xt_v[:, k, :c0w])
            for k in range(KD):
                nc.sync.dma_start(out=xt_sb[:, k, c0w:], in_=xt_v[:, k, c0w:])

            w1_v = w1t.rearrange("(k p) h -> p k h", p=P)
            w2_v = w2t.rearrange("(k p) h -> p k h", p=P)
            w3_v = w3t.rearrange("(k p) d -> p k d", p=P)

            for r in range(rounds):
                src = xt_sb if r % 2 == 0 else out_sb
                dst = out_sb if r % 2 == 0 else xt_sb

                # ---- phase 1: hidden = silu(x@W1^T) * (x@W2^T), [H, C] ----
                for h in range(KH):
                    w1_tile = wpool.tile([P, KD, P], F32R, tag="w1")
                    w2_tile = wpool.tile([P, KD, P], F32R, tag="w2")
                    nc.sync.dma_start(
                        out=w1_tile[:], in_=w1_v[:, :, h * P : (h + 1) * P]
                    )
                    nc.sync.dma_start(
                        out=w2_tile[:], in_=w2_v[:, :, h * P : (h + 1) * P]
                    )
                    for ci, cw in enumerate(cks):
                        c0 = coff[ci]
                        h1_ps = psum.tile([P, 512], F32, tag="h1")
                        h2_ps = psum.tile([P, 512], F32, tag="h2")
                        for k in range(KD):
                            nc.tensor.matmul(
                                out=h1_ps[:, :cw],
                                lhsT=w1_tile[:, k, :],
                                rhs=src[:, k, c0 : c0 + cw],
                                start=(k == 0),
                                stop=(k == KD - 1),
                            )
                        for k in range(KD):
                            nc.tensor.matmul(
                                out=h2_ps[:, :cw],
                                lhsT=w2_tile[:, k, :],
                                rhs=src[:, k, c0 : c0 + cw],
                                start=(k == 0),
                                stop=(k == KD - 1),
                            )
                        silu_sb = actp.tile([P, 512], F32, tag="silu")
                        nc.scalar.activation(
                            out=silu_sb[:, :cw],
                            in_=h1_ps[:, :cw],
                            func=mybir.ActivationFunctionType.Silu,
                        )
                        # copy h2 PSUM->SBUF on ACT (same engine as silu) so
                        # the DVE mul carries a single sync wait — walrus's TT
                        # descriptor rejects DVE ops with more than one wait
                        h2_sb = actp.tile([P, 512], F32, tag="h2sb")
                        nc.scalar.copy(out=h2_sb[:, :cw], in_=h2_ps[:, :cw])
                        nc.vector.tensor_mul(
                            out=hid_sb[:, h, c0 : c0 + cw],
                            in0=silu_sb[:, :cw],
                            in1=h2_sb[:, :cw],
                        )

                # ---- phase 2: y = hidden @ W3^T, [D, C] ----
                for d in range(KD):
                    w3_tile = wpool.tile([P, KH, P], F32R, tag="w3")
                    nc.sync.dma_start(
                        out=w3_tile[:], in_=w3_v[:, :, d * P : (d + 1) * P]
                    )
                    for ci, cw in enumerate(cks):
                        c0 = coff[ci]
                        y_ps = psum.tile([P, 512], F32, tag="y")
                        for h in range(KH):
                            nc.tensor.matmul(
                                out=y_ps[:, :cw],
                                lhsT=w3_tile[:, h, :],
                                rhs=hid_sb[:, h, c0 : c0 + cw],
                                start=(h == 0),
                                stop=(h == KH - 1),
                            )
                        nc.scalar.activation(
                            out=dst[:, d, c0 : c0 + cw],
                            in_=y_ps[:, :cw],
                            func=mybir.ActivationFunctionType.Copy,
                        )  # ACT copyback keeps DVE free for phase-1 tail
                    if r == rounds - 1:
                        nc.sync.dma_start(
                            out=yt[d * P : (d + 1) * P, :],
                            in_=dst[:, d, :].bitcast(F32),
                        )

    if not nc.is_finalized():
        nc.finalize()  # Bacc.compile(): splits multi-waits, allocates regs
    return nc


def _route(x2d, Wg):
    """Host gating: scores, full softmax, top-2, aux losses (float64)."""
    scores = (x2d @ Wg.T).astype(np.float64)  # [N, E]
    m = scores.max(-1, keepdims=True)
    ex = np.exp(scores - m)
    probs_full = ex / ex.sum(-1, keepdims=True)

    top_idx = np.argsort(-scores, axis=-1, kind="stable")[:, :TOP_K]  # [N, K]
    top_scores = np.take_along_axis(scores, top_idx, axis=-1)
    tm = top_scores.max(-1, keepdims=True)
    tex = np.exp(top_scores - tm)
    top_p = tex / tex.sum(-1, keepdims=True)  # [N, K]

    N = x2d.shape[0]
    importance = probs_full.mean(axis=0)  # [E]
    load = np.bincount(top_idx.ravel(), minlength=E) / (N * TOP_K)
    lb_loss = E * np.sum(importance * load)
    ent_loss = (probs_full * np.log(np.clip(probs_full, EPS, None))).sum(-1).mean()
    total = LB_COEF * lb_loss + ENT_COEF * ent_loss
    return top_idx, top_p.astype(np.float32), lb_loss, ent_loss, total


def kernel(x, Wg, W1, W2, W3):
    x = np.asarray(x, dtype=np.float32)
    Wg = np.asarray(Wg, dtype=np.float32)
    W1 = np.asarray(W1, dtype=np.float32)
    W2 = np.asarray(W2, dtype=np.float32)
    W3 = np.asarray(W3, dtype=np.float32)

    N = B * S
    x2d = x.reshape(N, D)
    top_idx, top_p, lb_loss, ent_loss, total = _route(x2d, Wg)

    tok_lists = [np.where((top_idx[:, 0] == e) | (top_idx[:, 1] == e))[0] for e in range(E)]
    counts = [len(t) for t in tok_lists]
    C = max(512, -(-max(counts) // 32) * 32)  # pad to multiple of 32, >= 512

    in_maps = []
    for e in range(E):
        toks = tok_lists[e]
        xe = np.zeros((C, D), dtype=np.float32)
        xe[: len(toks)] = x2d[toks]
        in_maps.append(
            {
                "xt": np.ascontiguousarray(xe.T),
                "w1t": np.ascontiguousarray(W1[e].T),
                "w2t": np.ascontiguousarray(W2[e].T),
                "w3t": np.ascontiguousarray(W3[e].T),
            }
        )

    global last_C
    last_C = C
    nc = build_ffn_kernel(C)
    trace = bool(os.environ.get("MOE_TRACE"))
    res = run_bass_kernel_spmd(
        nc, in_maps, core_ids=list(range(N_CORES)), trace=trace
    )
    global last_results
    last_results = res

    y = np.zeros((N, D), dtype=np.float32)
    for e in range(E):
        toks = tok_lists[e]
        out_e = res.results[e]["yt"][:, : len(toks)].T  # [n_e, D]
        w = np.where(top_idx[toks, 0] == e, top_p[toks, 0], top_p[toks, 1])
        y[toks] += w[:, None].astype(np.float32) * out_e

    return (
        y.reshape(B, S, D),
        np.float32(lb_loss),
        np.float32(ent_loss),
        np.float32(total),
    )


# revision 22
# speedup vs baseline: 1.0054x; 1.0054x over previous
"""MoE FFN with auxiliary loss — Trainium2 Bass kernel.

Strategy (expert-parallel, host-routed):
  The reference computes every expert on every token, but the combine
  weights are nonzero only for each token's top-2 experts — so only
  top-2 expert outputs are needed.  Gating/top-k/aux-losses are tiny
  (0.03% of FLOPs) and run on host.  Each of the 8 NeuronCores owns one
  expert (E=8): the host gathers that expert's routed tokens, the core
  runs the SwiGLU FFN  (silu(x@W1^T) * (x@W2^T)) @ W3^T  on them, and
  the host scales by gate probs and scatters back.

  Device layout: everything transposed so the token axis is the matmul
  free (moving) axis — PSUM partition = output-feature tiles:
    phase 1: H1T[h,c] += W1T[d-tile,h-tile]^T @ XT[d-tile,c]   (lhsT=W1T)
    silu/mul fused on ACT+DVE into hidden slab [h, c] in SBUF
    phase 2: YT[d,c]  += W3T[h-tile,d-tile]^T @ hidden[h-tile,c]
  float32r matmuls run at full PE rate (1 cycle/row) for free dim >=256.
"""

import os
import sys

sys.path.insert(0, "/opt/trn_rl_repo")

last_results = None  # BassKernelResults of the most recent run (for profiling)
last_C = None  # capacity used by the most recent run

import numpy as np

import concourse.bass as bass
from concourse import bacc
import concourse.mybir as mybir
from concourse.bass_utils import run_bass_kernel_spmd
from concourse.tile import TileContext

TOP_K = 2
EPS = 1e-9
LB_COEF = 0.01
ENT_COEF = 0.0

B, S, D, E, H = 2, 2048, 1024, 8, 2048
N_CORES = 8
P = 128

F32R = mybir.dt.float32r
F32 = mybir.dt.float32


def _free_chunks(C):
    """Split C (>=512) into free-dim chunks, each in [256, 512] so float32r
    matmuls stay at the 1-cycle/row rate and PSUM free-dim fits."""
    chunks = []
    r = C
    while r > 512:
        take = 512 if r - 512 >= 256 else r - 256
        chunks.append(take)
        r -= take
    chunks.append(r)
    assert sum(chunks) == C and all(256 <= c <= 512 for c in chunks), (C, chunks)
    return chunks


def build_ffn_kernel(C, rounds=1):
    """One expert's SwiGLU FFN over C routed tokens (token axis = free).

    rounds>1 (benchmarking only) re-applies the FFN to its own output via
    ping-pong SBUF slabs, so per-round HW time can be isolated from the
    fixed dispatch overhead: (wall(R) - wall(1)) / (R - 1).
    """
    nc = bacc.Bacc()

    xt = nc.declare_dram_parameter("xt", [D, C], F32R, isOutput=False)
    w1t = nc.declare_dram_parameter("w1t", [D, H], F32R, isOutput=False)
    w2t = nc.declare_dram_parameter("w2t", [D, H], F32R, isOutput=False)
    w3t = nc.declare_dram_parameter("w3t", [H, D], F32R, isOutput=False)
    yt = nc.declare_dram_parameter("yt", [D, C], F32, isOutput=True)

    KD = D // P  # 8 k-tiles over D
    KH = H // P  # 16 tiles over H
    cks = _free_chunks(C)
    coff = np.cumsum([0] + cks)[:-1]

    with TileContext(nc) as tc:
        with (
            tc.tile_pool(name="resident", bufs=1) as resident,
            tc.tile_pool(name="wpool", bufs=3 if rounds == 1 else 2) as wpool,
            tc.tile_pool(name="act", bufs=3) as actp,
            tc.tile_pool(name="psum", bufs=2, space="PSUM") as psum,
        ):
            # resident slabs; rounds==1 writes back into xt_sb (phase 2 only
            # starts after phase 1 is done reading it), saving a third slab
            xt_sb = resident.tile([P, KD, C], F32R, tag="xt_sb")
            hid_sb = resident.tile([P, KH, C], F32R, tag="hid_sb")
            if rounds > 1:
                out_sb = resident.tile([P, KD, C], F32R, tag="out_sb")
            else:
                out_sb = xt_sb

            # load X^T: (k p) c -> p k c, two stages — chunk-0 columns for
            # all k first so the first matmuls start early, then the rest
            # (fully chunk-granular loads measured slower: DMA op overhead)
            xt_v = xt.rearrange("(k p) c -> p k c", p=P)
            c0w = cks[0]
            for k in range(KD):
                nc.sync.dma_start(out=xt_sb[:, k, :c0w], in_=xt_v[:, k, :c0w])
            for k in range(KD):
                nc.sync.dma_start(out=xt_sb[:, k, c0w:], in_=xt_v[:, k, c0w:])

            w1_v = w1t.rearrange("(k p) h -> p k h", p=P)
            w2_v = w2t.rearrange("(k p) h -> p k h", p=P)
            w3_v = w3t.rearrange("(k p) d -> p k d", p=P)

            for r in range(rounds):
                src = xt_sb if r % 2 == 0 else out_sb
                dst = out_sb if r % 2 == 0 else xt_sb

                # ---- phase 1: hidden = silu(x@W1^T) * (x@W2^T), [H, C] ----
                for h in range(KH):
                    w1_tile = wpool.tile([P, KD, P], F32R, tag="w1")
                    w2_tile = wpool.tile([P, KD, P], F32R, tag="w2")
                    nc.sync.dma_start(
                        out=w1_tile[:], in_=w1_v[:, :, h * P : (h + 1) * P]
                    )
                    nc.sync.dma_start(
                        out=w2_tile[:], in_=w2_v[:, :, h * P : (h + 1) * P]
                    )
                    for ci, cw in enumerate(cks):
                        c0 = coff[ci]
                        h1_ps = psum.tile([P, 512], F32, tag="h1")
                        h2_ps = psum.tile([P, 512], F32, tag="h2")
                        for k in range(KD):
                            nc.tensor.matmul(
                                out=h1_ps[:, :cw],
                                lhsT=w1_tile[:, k, :],
                                rhs=src[:, k, c0 : c0 + cw],
                                start=(k == 0),
                                stop=(k == KD - 1),
                            )
                        for k in range(KD):
                            nc.tensor.matmul(
                                out=h2_ps[:, :cw],
                                lhsT=w2_tile[:, k, :],
                                rhs=src[:, k, c0 : c0 + cw],
                                start=(k == 0),
                                stop=(k == KD - 1),
                            )
                        silu_sb = actp.tile([P, 512], F32, tag="silu")
                        nc.scalar.activation(
                            out=silu_sb[:, :cw],
                            in_=h1_ps[:, :cw],
                            func=mybir.ActivationFunctionType.Silu,
                        )
                        # copy h2 PSUM->SBUF on ACT (same engine as silu) so
                        # the DVE mul carries a single sync wait — walrus's TT
                        # descriptor rejects DVE ops with more than one wait
                        h2_sb = actp.tile([P, 512], F32, tag="h2sb")
                        nc.scalar.copy(out=h2_sb[:, :cw], in_=h2_ps[:, :cw])
                        nc.vector.tensor_mul(
                            out=hid_sb[:, h, c0 : c0 + cw],
                            in0=silu_sb[:, :cw],
                            in1=h2_sb[:, :cw],
                        )

                # ---- phase 2: y = hidden @ W3^T, [D, C] ----
                for d in range(KD):
                    w3_tile = wpool.tile([P, KH, P], F32R, tag="w3")
                    nc.sync.dma_start(
                        out=w3_tile[:], in_=w3_v[:, :, d * P : (d + 1) * P]
                    )
                    for ci, cw in enumerate(cks):
                        c0 = coff[ci]
                        y_ps = psum.tile([P, 512], F32, tag="y")
                        for h in range(KH):
                            nc.tensor.matmul(
                                out=y_ps[:, :cw],
                                lhsT=w3_tile[:, h, :],
                                rhs=hid_sb[:, h, c0 : c0 + cw],
                                start=(h == 0),
                                stop=(h == KH - 1),
                            )
                        if rounds == 1:
                            # straight to DRAM per chunk — measured ~1 µs
                            # faster than staging the whole row in the slab
                            ow = actp.tile([P, 512], F32, tag="ow")
                            nc.scalar.activation(
                                out=ow[:, :cw],
                                in_=y_ps[:, :cw],
                                func=mybir.ActivationFunctionType.Copy,
                            )
                            nc.sync.dma_start(
                                out=yt[d * P : (d + 1) * P, c0 : c0 + cw],
                                in_=ow[:, :cw],
                            )
                        else:
                            nc.scalar.activation(
                                out=dst[:, d, c0 : c0 + cw],
                                in_=y_ps[:, :cw],
                                func=mybir.ActivationFunctionType.Copy,
                            )  # ACT copyback keeps DVE free for phase-1 tail
                    if r == rounds - 1 and rounds > 1:
                        nc.sync.dma_start(
                            out=yt[d * P : (d + 1) * P, :],
                            in_=dst[:, d, :].bitcast(F32),
                        )

    if not nc.is_finalized():
        nc.finalize()  # Bacc.compile(): splits multi-waits, allocates regs
    return nc


def _route(x2d, Wg):
    """Host gating: scores, full softmax, top-2, aux losses (float64)."""
    scores = (x2d @ Wg.T).astype(np.float64)  # [N, E]
    m = scores.max(-1, keepdims=True)
    ex = np.exp(scores - m)
    probs_full = ex / ex.sum(-1, keepdims=True)

    top_idx = np.argsort(-scores, axis=-1, kind="stable")[:, :TOP_K]  # [N, K]
    top_scores = np.take_along_axis(scores, top_idx, axis=-1)
    tm = top_scores.max(-1, keepdims=True)
    tex = np.exp(top_scores - tm)
    top_p = tex / tex.sum(-1, keepdims=True)  # [N, K]

    N = x2d.shape[0]
    importance = probs_full.mean(axis=0)  # [E]
    load = np.bincount(top_idx.ravel(), minlength=E) / (N * TOP_K)
    lb_loss = E * np.sum(importance * load)
    ent_loss = (probs_full * np.log(np.clip(probs_full, EPS, None))).sum(-1).mean()
    total = LB_COEF * lb_loss + ENT_COEF * ent_loss
    return top_idx, top_p.astype(np.float32), lb_loss, ent_loss, total


def kernel(x, Wg, W1, W2, W3):
    x = np.asarray(x, dtype=np.float32)
    Wg = np.asarray(Wg, dtype=np.float32)
    W1 = np.asarray(W1, dtype=np.float32)
    W2 = np.asarray(W2, dtype=np.float32)
    W3 = np.asarray(W3, dtype=np.float32)

    N = B * S
    x2d = x.reshape(N, D)
    top_idx, top_p, lb_loss, ent_loss, total = _route(x2d, Wg)

    tok_lists = [np.where((top_idx[:, 0] == e) | (top_idx[:, 1] == e))[0] for e in range(E)]
    counts = [len(t) for t in tok_lists]
    C = max(512, -(-max(counts) // 32) * 32)  # pad to multiple of 32, >= 512

    in_maps = []
    for e in range(E):
        toks = tok_lists[e]
        xe = np.zeros((C, D), dtype=np.float32)
        xe[: len(toks)] = x2d[toks]
        in_maps.append(
            {
                "xt": np.ascontiguousarray(xe.T),
                "w1t": np.ascontiguousarray(W1[e].T),
                "w2t": np.ascontiguousarray(W2[e].T),
                "w3t": np.ascontiguousarray(W3[e].T),
            }
        )

    global last_C
    last_C = C
    nc = build_ffn_kernel(C)
    trace = bool(os.environ.get("MOE_TRACE"))
    res = run_bass_kernel_spmd(
        nc, in_maps, core_ids=list(range(N_CORES)), trace=trace
    )
    global last_results
    last_results = res

    y = np.zeros((N, D), dtype=np.float32)
    for e in range(E):
        toks = tok_lists[e]
        out_e = res.results[e]["yt"][:, : len(toks)].T  # [n_e, D]
        w = np.where(top_idx[toks, 0] == e, top_p[toks, 0], top_p[toks, 1])
        y[toks] += w[:, None].astype(np.float32) * out_e

    return (
        y.reshape(B, S, D),
        np.float32(lb_loss),
        np.float32(ent_loss),
        np.float32(total),
    )


# revision 23
# speedup vs baseline: 1.0185x; 1.0130x over previous
"""MoE FFN with auxiliary loss — Trainium2 Bass kernel.

Strategy (expert-parallel, host-routed):
  The reference computes every expert on every token, but the combine
  weights are nonzero only for each token's top-2 experts — so only
  top-2 expert outputs are needed.  Gating/top-k/aux-losses are tiny
  (0.03% of FLOPs) and run on host.  Each of the 8 NeuronCores owns one
  expert (E=8): the host gathers that expert's routed tokens, the core
  runs the SwiGLU FFN  (silu(x@W1^T) * (x@W2^T)) @ W3^T  on them, and
  the host scales by gate probs and scatters back.

  Device layout: everything transposed so the token axis is the matmul
  free (moving) axis — PSUM partition = output-feature tiles:
    phase 1: H1T[h,c] += W1T[d-tile,h-tile]^T @ XT[d-tile,c]   (lhsT=W1T)
    silu/mul fused on ACT+DVE into hidden slab [h, c] in SBUF
    phase 2: YT[d,c]  += W3T[h-tile,d-tile]^T @ hidden[h-tile,c]
  float32r matmuls run at full PE rate (1 cycle/row) for free dim >=256.
"""

import os
import sys

sys.path.insert(0, "/opt/trn_rl_repo")

last_results = None  # BassKernelResults of the most recent run (for profiling)
last_C = None  # capacity used by the most recent run

import numpy as np

import concourse.bass as bass
from concourse import bacc
import concourse.mybir as mybir
from concourse.bass_utils import run_bass_kernel_spmd
from concourse.tile import TileContext

TOP_K = 2
EPS = 1e-9
LB_COEF = 0.01
ENT_COEF = 0.0

B, S, D, E, H = 2, 2048, 1024, 8, 2048
N_CORES = 8
P = 128

F32R = mybir.dt.float32r
F32 = mybir.dt.float32


def _free_chunks(C):
    """Split C (>=512) into free-dim chunks, each in [256, 512] so float32r
    matmuls stay at the 1-cycle/row rate and PSUM free-dim fits."""
    chunks = []
    r = C
    while r > 512:
        take = 512 if r - 512 >= 256 else r - 256
        chunks.append(take)
        r -= take
    chunks.append(r)
    assert sum(chunks) == C and all(256 <= c <= 512 for c in chunks), (C, chunks)
    return chunks


def build_ffn_kernel(C, rounds=1):
    """One expert's SwiGLU FFN over C routed tokens (token axis = free).

    rounds>1 (benchmarking only) re-applies the FFN to its own output via
    ping-pong SBUF slabs, so per-round HW time can be isolated from the
    fixed dispatch overhead: (wall(R) - wall(1)) / (R - 1).
    """
    nc = bacc.Bacc()

    xt = nc.declare_dram_parameter("xt", [D, C], F32R, isOutput=False)
    w1t = nc.declare_dram_parameter("w1t", [D, H], F32R, isOutput=False)
    w2t = nc.declare_dram_parameter("w2t", [D, H], F32R, isOutput=False)
    w3t = nc.declare_dram_parameter("w3t", [H, D], F32R, isOutput=False)
    yt = nc.declare_dram_parameter("yt", [D, C], F32, isOutput=True)

    KD = D // P  # 8 k-tiles over D
    KH = H // P  # 16 tiles over H
    cks = _free_chunks(C)
    coff = np.cumsum([0] + cks)[:-1]

    with TileContext(nc) as tc:
        with (
            tc.tile_pool(name="resident", bufs=1) as resident,
            tc.tile_pool(name="wpool", bufs=3 if rounds == 1 else 2) as wpool,
            tc.tile_pool(name="act", bufs=3) as actp,
            tc.tile_pool(name="psum", bufs=2, space="PSUM") as psum,
        ):
            # resident slabs; rounds==1 writes back into xt_sb (phase 2 only
            # starts after phase 1 is done reading it), saving a third slab
            xt_sb = resident.tile([P, KD, C], F32R, tag="xt_sb")
            hid_sb = resident.tile([P, KH, C], F32R, tag="hid_sb")
            if rounds > 1:
                out_sb = resident.tile([P, KD, C], F32R, tag="out_sb")
            else:
                out_sb = xt_sb

            # load X^T: (k p) c -> p k c, two stages — chunk-0 columns for
            # all k first so the first matmuls start early, then the rest
            # (fully chunk-granular loads measured slower: DMA op overhead)
            xt_v = xt.rearrange("(k p) c -> p k c", p=P)
            c0w = cks[0]
            for k in range(KD):
                nc.sync.dma_start(out=xt_sb[:, k, :c0w], in_=xt_v[:, k, :c0w])
            for k in range(KD):
                nc.sync.dma_start(out=xt_sb[:, k, c0w:], in_=xt_v[:, k, c0w:])

            w1_v = w1t.rearrange("(k p) h -> p k h", p=P)
            w2_v = w2t.rearrange("(k p) h -> p k h", p=P)
            w3_v = w3t.rearrange("(k p) d -> p k d", p=P)

            for r in range(rounds):
                src = xt_sb if r % 2 == 0 else out_sb
                dst = out_sb if r % 2 == 0 else xt_sb

                # ---- phase 1: hidden = silu(x@W1^T) * (x@W2^T), [H, C] ----
                for h in range(KH):
                    w1_tile = wpool.tile([P, KD, P], F32R, tag="w1")
                    w2_tile = wpool.tile([P, KD, P], F32R, tag="w2")
                    nc.sync.dma_start(
                        out=w1_tile[:], in_=w1_v[:, :, h * P : (h + 1) * P]
                    )
                    nc.sync.dma_start(
                        out=w2_tile[:], in_=w2_v[:, :, h * P : (h + 1) * P]
                    )
                    for ci, cw in enumerate(cks):
                        c0 = coff[ci]
                        h1_ps = psum.tile([P, 512], F32, tag="h1")
                        h2_ps = psum.tile([P, 512], F32, tag="h2")
                        for k in range(KD):
                            nc.tensor.matmul(
                                out=h1_ps[:, :cw],
                                lhsT=w1_tile[:, k, :],
                                rhs=src[:, k, c0 : c0 + cw],
                                start=(k == 0),
                                stop=(k == KD - 1),
                            )
                        for k in range(KD):
                            nc.tensor.matmul(
                                out=h2_ps[:, :cw],
                                lhsT=w2_tile[:, k, :],
                                rhs=src[:, k, c0 : c0 + cw],
                                start=(k == 0),
                                stop=(k == KD - 1),
                            )
                        silu_sb = actp.tile([P, 512], F32, tag="silu")
                        nc.scalar.activation(
                            out=silu_sb[:, :cw],
                            in_=h1_ps[:, :cw],
                            func=mybir.ActivationFunctionType.Silu,
                        )
                        # copy h2 PSUM->SBUF on ACT (same engine as silu) so
                        # the DVE mul carries a single sync wait — walrus's TT
                        # descriptor rejects DVE ops with more than one wait
                        h2_sb = actp.tile([P, 512], F32, tag="h2sb")
                        nc.scalar.copy(out=h2_sb[:, :cw], in_=h2_ps[:, :cw])
                        nc.vector.tensor_mul(
                            out=hid_sb[:, h, c0 : c0 + cw],
                            in0=silu_sb[:, :cw],
                            in1=h2_sb[:, :cw],
                        )

                # ---- phase 2: y = hidden @ W3^T, [D, C] ----
                for d in range(KD):
                    w3_tile = wpool.tile([P, KH, P], F32R, tag="w3")
                    nc.sync.dma_start(
                        out=w3_tile[:], in_=w3_v[:, :, d * P : (d + 1) * P]
                    )
                    for ci, cw in enumerate(cks):
                        c0 = coff[ci]
                        y_ps = psum.tile([P, 512], F32, tag="y")
                        for h in range(KH):
                            nc.tensor.matmul(
                                out=y_ps[:, :cw],
                                lhsT=w3_tile[:, h, :],
                                rhs=hid_sb[:, h, c0 : c0 + cw],
                                start=(h == 0),
                                stop=(h == KH - 1),
                            )
                        if rounds == 1:
                            # straight to DRAM per chunk — measured ~1 µs
                            # faster than staging the whole row in the slab
                            ow = actp.tile([P, 512], F32, tag="ow")
                            nc.scalar.activation(
                                out=ow[:, :cw],
                                in_=y_ps[:, :cw],
                                func=mybir.ActivationFunctionType.Copy,
                            )
                            nc.sync.dma_start(
                                out=yt[d * P : (d + 1) * P, c0 : c0 + cw],
                                in_=ow[:, :cw],
                            )
                        else:
                            nc.scalar.activation(
                                out=dst[:, d, c0 : c0 + cw],
                                in_=y_ps[:, :cw],
                                func=mybir.ActivationFunctionType.Copy,
                            )  # ACT copyback keeps DVE free for phase-1 tail
                    if r == rounds - 1 and rounds > 1:
                        nc.sync.dma_start(
                            out=yt[d * P : (d + 1) * P, :],
                            in_=dst[:, d, :].bitcast(F32),
                        )

    if not nc.is_finalized():
        nc.finalize()  # Bacc.compile(): splits multi-waits, allocates regs
    return nc


def _route(x2d, Wg):
    """Host gating: scores, full softmax, top-2, aux losses (float64)."""
    scores = (x2d @ Wg.T).astype(np.float64)  # [N, E]
    m = scores.max(-1, keepdims=True)
    ex = np.exp(scores - m)
    probs_full = ex / ex.sum(-1, keepdims=True)

    top_idx = np.argsort(-scores, axis=-1, kind="stable")[:, :TOP_K]  # [N, K]
    top_scores = np.take_along_axis(scores, top_idx, axis=-1)
    tm = top_scores.max(-1, keepdims=True)
    tex = np.exp(top_scores - tm)
    top_p = tex / tex.sum(-1, keepdims=True)  # [N, K]

    N = x2d.shape[0]
    importance = probs_full.mean(axis=0)  # [E]
    load = np.bincount(top_idx.ravel(), minlength=E) / (N * TOP_K)
    lb_loss = E * np.sum(importance * load)
    ent_loss = (probs_full * np.log(np.clip(probs_full, EPS, None))).sum(-1).mean()
    total = LB_COEF * lb_loss + ENT_COEF * ent_loss
    return top_idx, top_p.astype(np.float32), lb_loss, ent_loss, total


def kernel(x, Wg, W1, W2, W3):
    x = np.asarray(x, dtype=np.float32)
    Wg = np.asarray(Wg, dtype=np.float32)
    W1 = np.asarray(W1, dtype=np.float32)
    W2 = np.asarray(W2, dtype=np.float32)
    W3 = np.asarray(W3, dtype=np.float32)

    N = B * S
    x2d = x.reshape(N, D)
    top_idx, top_p, lb_loss, ent_loss, total = _route(x2d, Wg)

    tok_lists = [np.where((top_idx[:, 0] == e) | (top_idx[:, 1] == e))[0] for e in range(E)]
    counts = [len(t) for t in tok_lists]
    C = max(512, -(-max(counts) // 16) * 16)  # pad to multiple of 16, >= 512

    in_maps = []
    for e in range(E):
        toks = tok_lists[e]
        xe = np.zeros((C, D), dtype=np.float32)
        xe[: len(toks)] = x2d[toks]
        in_maps.append(
            {
                "xt": np.ascontiguousarray(xe.T),
                "w1t": np.ascontiguousarray(W1[e].T),
                "w2t": np.ascontiguousarray(W2[e].T),
                "w3t": np.ascontiguousarray(W3[e].T),
            }
        )

    global last_C
    last_C = C
    nc = build_ffn_kernel(C)
    trace = bool(os.environ.get("MOE_TRACE"))
    res = run_bass_kernel_spmd(
        nc, in_maps, core_ids=list(range(N_CORES)), trace=trace
    )
    global last_results
    last_results = res

    y = np.zeros((N, D), dtype=np.float32)
    for e in range(E):
        toks = tok_lists[e]
        out_e = res.results[e]["yt"][:, : len(toks)].T  # [n_e, D]
        w = np.where(top_idx[toks, 0] == e, top_p[toks, 0], top_p[toks, 1])
        y[toks] += w[:, None].astype(np.float32) * out_e

    return (
        y.reshape(B, S, D),
        np.float32(lb_loss),
        np.float32(ent_loss),
        np.float32(total),
    )
